# revision 7
# baseline (speedup 1.0000x reference)
"""Causal attention (DS_FullAttention) Trainium2 Bass kernel.

Problem: B=4, H=8, L=S=2048, E=64 causal attention with a per-batch
exp(tau) de-stationarization rescale, fp32 I/O.

Sharding: the 32 (b, h) pairs are independent; each of the 8 cores gets 4
pairs. Inside a core, per pair:
  - scoresT[s, q] = K^T Q computed transposed (s on PSUM partitions) so the
    A @ V contraction (over s) needs no on-chip transpose of A.
  - exp applied psum->SBUF on the scalar engine with the per-pair scale
    folded into the ACTIVATE scale operand (softmax max-subtraction is
    skipped; scores are O(10) so fp32/fp16 exp is safe).
  - V carries an appended ones column, so the A @ V matmul also produces
    the softmax denominators in PSUM partition 64.
  - causal masking: block-level (never compute s-chunks above the
    diagonal) + a triangular fp16 mask multiply on diagonal 128x128 chunks.
  - normalization: denominators are DMA-reshaped to [128, 16], inverted
    with reciprocal_approx_accurate (2 ULP), broadcast back via a DRAM
    round trip, and applied with one tensor_tensor multiply per pair.

Matmuls run in fp16 (measured end-to-end error ~5e-4 vs the fp32
reference; fp16 hides PE weight loads, fp32/f32r cannot).
"""

import sys

if "/opt/trn_rl_repo" not in sys.path:
    sys.path.insert(0, "/opt/trn_rl_repo")

import numpy as np

import concourse.bass as bass
import concourse.mybir as mybir
import concourse.tile as tile
from concourse import bacc, bass_utils

B, L, S, H, E = 4, 2048, 2048, 8, 64
P = 128
NCORES = 8
PAIRS_PER_CORE = (B * H) // NCORES  # 4
NQB = L // 512  # 4 q-superblocks of 512
NSC = S // P  # 16 s-chunks of 128
E2 = E + 1  # V plus ones column
EXP_GROUP = 3  # s-chunks exp'd per ACTIVATE (3 PSUM banks)

f32 = mybir.dt.float32
fp16 = mybir.dt.float16
Exp = mybir.ActivationFunctionType.Exp
Mult = mybir.AluOpType.mult

_PROGRAM_CACHE = {}


def _build_program():
    if "nc" in _PROGRAM_CACHE:
        return _PROGRAM_CACHE["nc"]

    nc = bacc.Bacc(
        "TRN2",
        target_bir_lowering=False,
        debug=False,
        enable_asserts=False,
        num_devices=NCORES,
    )
    qt_d = nc.dram_tensor("qt", [PAIRS_PER_CORE, P, L], fp16, kind="ExternalInput")
    kt_d = nc.dram_tensor("kt", [PAIRS_PER_CORE, P, L], fp16, kind="ExternalInput")
    vp_d = nc.dram_tensor(
        "vp", [PAIRS_PER_CORE, P, NSC, E2], fp16, kind="ExternalInput"
    )
    tri_d = nc.dram_tensor("tri", [P, P], fp16, kind="ExternalInput")
    scl_d = nc.dram_tensor("scl", [P, PAIRS_PER_CORE], f32, kind="ExternalInput")
    o_d = nc.dram_tensor("o", [PAIRS_PER_CORE, E, L], f32, kind="ExternalOutput")

    with tile.TileContext(nc) as tc:
        with (
            tc.tile_pool(name="const", bufs=1) as const,
            tc.tile_pool(name="qk", bufs=2) as qk,
            tc.tile_pool(name="atp", bufs=4) as atp,
            tc.tile_pool(name="stg", bufs=3) as stg,
            tc.tile_pool(name="psS", bufs=2, space="PSUM") as psS,
            tc.tile_pool(name="psO", bufs=2, space="PSUM") as psO,
            tc.tile_pool(name="dram", bufs=2, space="DRAM") as dram,
        ):
            # warm-up exp: pulls the ~2.7us ACT table load under the DMAs
            wu = const.tile([P, 16], f32)
            nc.gpsimd.memset(wu[:], 0.0)
            nc.scalar.activation(wu[:], wu[:], Exp, scale=1.0)

            tri_t = const.tile([P, P], fp16)
            nc.sync.dma_start(tri_t[:], tri_d[:])
            scl_t = const.tile([P, PAIRS_PER_CORE], f32)
            nc.sync.dma_start(scl_t[:], scl_d[:])

            # mm2 + block epilogues run one exp-group behind (RAW decoupling)
            pending = [None]

            def flush():
                if pending[0] is not None:
                    pending[0]()
                    pending[0] = None

            for p in range(PAIRS_PER_CORE):
                qt_t = qk.tile([P, L], fp16, tag="qt")
                nc.sync.dma_start(qt_t[:], qt_d[p])
                kt_t = qk.tile([P, L], fp16, tag="kt")
                nc.sync.dma_start(kt_t[:], kt_d[p])
                vp_t = qk.tile([P, NSC, E2], fp16, tag="vp")
                nc.sync.dma_start(vp_t[:], vp_d[p])

                for iq in range(NQB):
                    q0 = 512 * iq
                    njs = 4 * iq + 4  # s-chunks this q-superblock touches
                    po = psO.tile([P, 512], f32, tag="po")

                    def mk_mm2(js, at, po=po, vp_t=vp_t, iq=iq, njs=njs,
                               p=p, q0=q0):
                        def emit():
                            for idx, j in enumerate(js):
                                qoff = max(0, P * (j - 4 * iq))
                                nc.tensor.matmul(
                                    po[0:E2, qoff:512],
                                    lhsT=vp_t[:, j, :],
                                    rhs=at[:, idx, qoff:512],
                                    start=(j == 0),
                                    stop=(j == njs - 1),
                                )
                            if js[-1] == njs - 1:
                                # q-superblock done: stage, invert sums,
                                # broadcast, normalize, store — all per-block
                                outU = stg.tile([E2, 512], f32, tag="outU")
                                nc.vector.tensor_copy(outU[:], po[0:E2, :])
                                d1q = dram.tile([1, 512], f32, tag="d1q")
                                nc.sync.dma_start(d1q[:], outU[E : E + 1, :])
                                ssq = stg.tile([P, 4], f32, tag="ssq")
                                nc.sync.dma_start(
                                    ssq[:],
                                    d1q[:].rearrange("o (r f) -> (o r) f", r=P),
                                )
                                rrq = stg.tile([P, 4], f32, tag="rrq")
                                rscr = stg.tile([P, 4], f32, tag="rscr")
                                nc.vector.reciprocal_approx_accurate(
                                    rrq[:], ssq[:], rscr[:]
                                )
                                d2q = dram.tile([1, 512], f32, tag="d2q")
                                nc.sync.dma_start(
                                    d2q[:].rearrange("o (r f) -> (o r) f", r=P),
                                    rrq[:],
                                )
                                r64 = stg.tile([E, 512], f32, tag="r64")
                                nc.sync.dma_start(
                                    r64[:], d2q[:].to_broadcast((E, 512))
                                )
                                outF = stg.tile([E, 512], f32, tag="outF")
                                nc.vector.tensor_tensor(
                                    outF[:], outU[0:E, :], r64[:], Mult
                                )
                                nc.sync.dma_start(
                                    o_d[p, :, q0 : q0 + 512], outF[:]
                                )

                        return emit

                    for g0 in range(0, njs, EXP_GROUP):
                        js = list(range(g0, min(g0 + EXP_GROUP, njs)))
                        ng = len(js)
                        ps = psS.tile([P, EXP_GROUP, 512], f32, tag="ps")
                        for idx, j in enumerate(js):
                            row = 64 * (j % 2)  # alternate row groups: LDW hides
                            qoff = max(0, P * (j - 4 * iq))
                            nc.tensor.matmul(
                                ps[:, idx, qoff:512],
                                lhsT=kt_t[row : row + 64, P * j : P * (j + 1)],
                                rhs=qt_t[row : row + 64, q0 + qoff : q0 + 512],
                                start=True,
                                stop=True,
                            )
                        at = atp.tile([P, EXP_GROUP, 512], fp16, tag="at")
                        # exp whole group; skip columns no chunk needs
                        qmin = min(max(0, P * (j - 4 * iq)) for j in js)
                        nc.scalar.activation(
                            at[:, :ng, qmin:512],
                            ps[:, :ng, qmin:512],
                            Exp,
                            scale=scl_t[:, p : p + 1],
                        )
                        for idx, j in enumerate(js):
                            d = j - 4 * iq
                            if d >= 0:  # diagonal chunk: triangular mask
                                qo = P * d
                                nc.vector.tensor_tensor(
                                    at[:, idx, qo : qo + P],
                                    at[:, idx, qo : qo + P],
                                    tri_t[:],
                                    Mult,
                                )
                        flush()
                        pending[0] = mk_mm2(js, at)
            flush()

    nc.compile()
    _PROGRAM_CACHE["nc"] = nc
    return nc


def _prep_core_inputs(queries, keys, values, tau, core):
    qt = np.empty((PAIRS_PER_CORE, P, L), dtype=np.float16)
    kt = np.empty((PAIRS_PER_CORE, P, L), dtype=np.float16)
    vp = np.zeros((PAIRS_PER_CORE, P, NSC, E2), dtype=np.float16)
    scl = np.empty((P, PAIRS_PER_CORE), dtype=np.float32)
    for p in range(PAIRS_PER_CORE):
        idx = PAIRS_PER_CORE * core + p
        b, h = divmod(idx, H)
        qT = np.ascontiguousarray(queries[b, :, h, :].T).astype(np.float16)  # [E, L]
        kT = np.ascontiguousarray(keys[b, :, h, :].T).astype(np.float16)
        qt[p, 0:E] = qT
        qt[p, E:P] = qT
        kt[p, 0:E] = kT
        kt[p, E:P] = kT
        # vp[p, si, so, e] = V[b, 128*so + si, h, e]; ones in column E
        vv = values[b, :, h, :].reshape(NSC, P, E).transpose(1, 0, 2)
        vp[p, :, :, 0:E] = vv.astype(np.float16)
        vp[p, :, :, E] = 1.0
        scl[:, p] = np.exp(tau[b, 0, 0, 0]) / np.sqrt(E)
    tri = np.triu(np.ones((P, P), dtype=np.float16))  # tri[s, q] = 1 iff s <= q
    return {"qt": qt, "kt": kt, "vp": vp, "tri": tri, "scl": scl}


def _run(inputs, trace=False):
    queries = np.asarray(inputs["queries"], dtype=np.float32)
    keys = np.asarray(inputs["keys"], dtype=np.float32)
    values = np.asarray(inputs["values"], dtype=np.float32)
    tau = np.asarray(inputs["tau"], dtype=np.float32)

    nc = _build_program()
    in_maps = [
        _prep_core_inputs(queries, keys, values, tau, c) for c in range(NCORES)
    ]
    res = bass_utils.run_bass_kernel_spmd(
        nc, in_maps, core_ids=list(range(NCORES)), trace=trace
    )
    out = np.empty((B, L, H, E), dtype=np.float32)
    for c in range(NCORES):
        o = res.results[c]["o"]  # [PAIRS, E, L]
        for p in range(PAIRS_PER_CORE):
            idx = PAIRS_PER_CORE * c + p
            b, h = divmod(idx, H)
            out[b, :, h, :] = o[p].T
    return out, res


def kernel(queries, keys, values, attn_mask, tau):
    out, _ = _run(
        {"queries": queries, "keys": keys, "values": values, "tau": tau},
        trace=False,
    )
    return out


def kernel_traced(queries, keys, values, attn_mask, tau):
    out, res = _run(
        {"queries": queries, "keys": keys, "values": values, "tau": tau},
        trace=True,
    )
    return out, res


# revision 8
# speedup vs baseline: 1.1441x; 1.1441x over previous
"""Causal attention (DS_FullAttention) Trainium2 Bass kernel.

Problem: B=4, H=8, L=S=2048, E=64 causal attention with a per-batch
exp(tau) de-stationarization rescale, fp32 I/O.

Sharding: the 32 (b, h) pairs are independent; each of the 8 cores gets 4
pairs. Inside a core, per pair:
  - scoresT[s, q] = K^T Q computed transposed (s on PSUM partitions) so the
    A @ V contraction (over s) needs no on-chip transpose of A.
  - exp applied psum->SBUF on the scalar engine with the per-pair scale
    folded into the ACTIVATE scale operand (softmax max-subtraction is
    skipped; scores are O(10) so fp32/fp16 exp is safe).
  - V carries an appended ones column, so the A @ V matmul also produces
    the softmax denominators in PSUM partition 64.
  - causal masking: block-level (never compute s-chunks above the
    diagonal) + a triangular fp16 mask multiply on diagonal 128x128 chunks.
  - normalization: denominators are DMA-reshaped to [128, 16], inverted
    with reciprocal_approx_accurate (2 ULP), broadcast back via a DRAM
    round trip, and applied with one tensor_tensor multiply per pair.

Matmuls run in fp16 (measured end-to-end error ~5e-4 vs the fp32
reference; fp16 hides PE weight loads, fp32/f32r cannot).
"""

import sys

if "/opt/trn_rl_repo" not in sys.path:
    sys.path.insert(0, "/opt/trn_rl_repo")

import numpy as np

import concourse.bass as bass
import concourse.mybir as mybir
import concourse.tile as tile
from concourse import bacc, bass_utils

B, L, S, H, E = 4, 2048, 2048, 8, 64
P = 128
NCORES = 8
PAIRS_PER_CORE = (B * H) // NCORES  # 4
NQB = L // 512  # 4 q-superblocks of 512
NSC = S // P  # 16 s-chunks of 128
E2 = E + 1  # V plus ones column
EXP_GROUP = 3  # s-chunks exp'd per ACTIVATE (3 PSUM banks)

f32 = mybir.dt.float32
fp16 = mybir.dt.float16
Exp = mybir.ActivationFunctionType.Exp
Mult = mybir.AluOpType.mult

_PROGRAM_CACHE = {}


def _build_program():
    if "nc" in _PROGRAM_CACHE:
        return _PROGRAM_CACHE["nc"]

    nc = bacc.Bacc(
        "TRN2",
        target_bir_lowering=False,
        debug=False,
        enable_asserts=False,
        num_devices=NCORES,
    )
    qt_d = nc.dram_tensor("qt", [PAIRS_PER_CORE, P, L], fp16, kind="ExternalInput")
    kt_d = nc.dram_tensor("kt", [PAIRS_PER_CORE, P, L], fp16, kind="ExternalInput")
    vp_d = nc.dram_tensor(
        "vp", [PAIRS_PER_CORE, P, NSC, E2], fp16, kind="ExternalInput"
    )
    tri_d = nc.dram_tensor("tri", [P, P], fp16, kind="ExternalInput")
    scl_d = nc.dram_tensor("scl", [P, PAIRS_PER_CORE], f32, kind="ExternalInput")
    o_d = nc.dram_tensor("o", [PAIRS_PER_CORE, E, L], f32, kind="ExternalOutput")

    with tile.TileContext(nc) as tc:
        with (
            tc.tile_pool(name="const", bufs=1) as const,
            tc.tile_pool(name="qk", bufs=2) as qk,
            tc.tile_pool(name="atp", bufs=4) as atp,
            tc.tile_pool(name="stg", bufs=3) as stg,
            tc.tile_pool(name="psS", bufs=2, space="PSUM") as psS,
            tc.tile_pool(name="psO", bufs=2, space="PSUM") as psO,
            tc.tile_pool(name="dram", bufs=2, space="DRAM") as dram,
        ):
            # warm-up exp: pulls the ~2.7us ACT table load under the DMAs
            wu = const.tile([P, 16], f32)
            nc.gpsimd.memset(wu[:], 0.0)
            nc.scalar.activation(wu[:], wu[:], Exp, scale=1.0)

            tri_t = const.tile([P, P], fp16)
            nc.sync.dma_start(tri_t[:], tri_d[:])
            scl_t = const.tile([P, PAIRS_PER_CORE], f32)
            nc.sync.dma_start(scl_t[:], scl_d[:])

            # mm2 + block epilogues run one exp-group behind (RAW decoupling)
            pending = [None]

            def flush():
                if pending[0] is not None:
                    pending[0]()
                    pending[0] = None

            for p in range(PAIRS_PER_CORE):
                qt_t = qk.tile([P, L], fp16, tag="qt")
                nc.sync.dma_start(qt_t[:], qt_d[p])
                kt_t = qk.tile([P, L], fp16, tag="kt")
                nc.sync.dma_start(kt_t[:], kt_d[p])
                vp_t = qk.tile([P, NSC, E2], fp16, tag="vp")
                nc.sync.dma_start(vp_t[:], vp_d[p])

                # unnormalized output + denominators, staged per pair
                outU = stg.tile([E2, NQB, 512], f32, tag="outU")
                d1 = dram.tile([1, L], f32, tag="d1")

                for iq in range(NQB):
                    q0 = 512 * iq
                    njs = 4 * iq + 4  # s-chunks this q-superblock touches
                    po = psO.tile([P, 512], f32, tag="po")

                    def mk_mm2(js, at, po=po, vp_t=vp_t, iq=iq, njs=njs,
                               p=p, q0=q0, outU=outU, d1=d1):
                        def emit():
                            for idx, j in enumerate(js):
                                qoff = max(0, P * (j - 4 * iq))
                                nc.tensor.matmul(
                                    po[0:E2, qoff:512],
                                    lhsT=vp_t[:, j, :],
                                    rhs=at[:, idx, qoff:512],
                                    start=(j == 0),
                                    stop=(j == njs - 1),
                                )
                            if js[-1] != njs - 1:
                                return
                            # q-superblock done: stage + ship denominators
                            nc.vector.tensor_copy(outU[:, iq, :], po[0:E2, :])
                            nc.gpsimd.dma_start(
                                d1[0:1, q0 : q0 + 512], outU[E : E + 1, iq, :]
                            )
                            if iq != NQB - 1:
                                return
                            # pair done: invert sums via [128, 16] reshape,
                            # broadcast via DRAM, normalize, store
                            ss = stg.tile([P, L // P], f32, tag="ss")
                            nc.gpsimd.dma_start(
                                ss[:],
                                d1[:].rearrange("o (r f) -> (o r) f", r=P),
                            )
                            rr = stg.tile([P, L // P], f32, tag="rr")
                            rscr = stg.tile([P, L // P], f32, tag="rscr")
                            nc.vector.reciprocal_approx_accurate(
                                rr[:], ss[:], rscr[:]
                            )
                            d2 = dram.tile([1, L], f32, tag="d2")
                            nc.gpsimd.dma_start(
                                d2[:].rearrange("o (r f) -> (o r) f", r=P),
                                rr[:],
                            )
                            r64 = stg.tile([E, L], f32, tag="r64")
                            nc.gpsimd.dma_start(
                                r64[:], d2[:].to_broadcast((E, L))
                            )
                            outF = stg.tile([E, L], f32, tag="outF")
                            nc.vector.tensor_tensor(
                                outF[:],
                                outU[0:E, :, :].rearrange("e b q -> e (b q)"),
                                r64[:],
                                Mult,
                            )
                            nc.sync.dma_start(o_d[p], outF[:])

                        return emit

                    for g0 in range(0, njs, EXP_GROUP):
                        js = list(range(g0, min(g0 + EXP_GROUP, njs)))
                        ng = len(js)
                        ps = psS.tile([P, EXP_GROUP, 512], f32, tag="ps")
                        for idx, j in enumerate(js):
                            row = 64 * (j % 2)  # alternate row groups: LDW hides
                            qoff = max(0, P * (j - 4 * iq))
                            nc.tensor.matmul(
                                ps[:, idx, qoff:512],
                                lhsT=kt_t[row : row + 64, P * j : P * (j + 1)],
                                rhs=qt_t[row : row + 64, q0 + qoff : q0 + 512],
                                start=True,
                                stop=True,
                            )
                        at = atp.tile([P, EXP_GROUP, 512], fp16, tag="at")
                        # exp whole group; skip columns no chunk needs
                        qmin = min(max(0, P * (j - 4 * iq)) for j in js)
                        nc.scalar.activation(
                            at[:, :ng, qmin:512],
                            ps[:, :ng, qmin:512],
                            Exp,
                            scale=scl_t[:, p : p + 1],
                        )
                        for idx, j in enumerate(js):
                            d = j - 4 * iq
                            if d >= 0:  # diagonal chunk: triangular mask
                                qo = P * d
                                nc.vector.tensor_tensor(
                                    at[:, idx, qo : qo + P],
                                    at[:, idx, qo : qo + P],
                                    tri_t[:],
                                    Mult,
                                )
                        flush()
                        pending[0] = mk_mm2(js, at)
            flush()

    nc.compile()
    _PROGRAM_CACHE["nc"] = nc
    return nc


def _prep_core_inputs(queries, keys, values, tau, core):
    qt = np.empty((PAIRS_PER_CORE, P, L), dtype=np.float16)
    kt = np.empty((PAIRS_PER_CORE, P, L), dtype=np.float16)
    vp = np.zeros((PAIRS_PER_CORE, P, NSC, E2), dtype=np.float16)
    scl = np.empty((P, PAIRS_PER_CORE), dtype=np.float32)
    for p in range(PAIRS_PER_CORE):
        idx = PAIRS_PER_CORE * core + p
        b, h = divmod(idx, H)
        qT = np.ascontiguousarray(queries[b, :, h, :].T).astype(np.float16)  # [E, L]
        kT = np.ascontiguousarray(keys[b, :, h, :].T).astype(np.float16)
        qt[p, 0:E] = qT
        qt[p, E:P] = qT
        kt[p, 0:E] = kT
        kt[p, E:P] = kT
        # vp[p, si, so, e] = V[b, 128*so + si, h, e]; ones in column E
        vv = values[b, :, h, :].reshape(NSC, P, E).transpose(1, 0, 2)
        vp[p, :, :, 0:E] = vv.astype(np.float16)
        vp[p, :, :, E] = 1.0
        scl[:, p] = np.exp(tau[b, 0, 0, 0]) / np.sqrt(E)
    tri = np.triu(np.ones((P, P), dtype=np.float16))  # tri[s, q] = 1 iff s <= q
    return {"qt": qt, "kt": kt, "vp": vp, "tri": tri, "scl": scl}


def _run(inputs, trace=False):
    queries = np.asarray(inputs["queries"], dtype=np.float32)
    keys = np.asarray(inputs["keys"], dtype=np.float32)
    values = np.asarray(inputs["values"], dtype=np.float32)
    tau = np.asarray(inputs["tau"], dtype=np.float32)

    nc = _build_program()
    in_maps = [
        _prep_core_inputs(queries, keys, values, tau, c) for c in range(NCORES)
    ]
    res = bass_utils.run_bass_kernel_spmd(
        nc, in_maps, core_ids=list(range(NCORES)), trace=trace
    )
    out = np.empty((B, L, H, E), dtype=np.float32)
    for c in range(NCORES):
        o = res.results[c]["o"]  # [PAIRS, E, L]
        for p in range(PAIRS_PER_CORE):
            idx = PAIRS_PER_CORE * c + p
            b, h = divmod(idx, H)
            out[b, :, h, :] = o[p].T
    return out, res


def kernel(queries, keys, values, attn_mask, tau):
    out, _ = _run(
        {"queries": queries, "keys": keys, "values": values, "tau": tau},
        trace=False,
    )
    return out


def kernel_traced(queries, keys, values, attn_mask, tau):
    out, res = _run(
        {"queries": queries, "keys": keys, "values": values, "tau": tau},
        trace=True,
    )
    return out, res
